# revision 11
# baseline (speedup 1.0000x reference)
"""EpsSupInfoNCE loss on 8 Trainium2 NeuronCores — v3.

Math (reference): logits = (E @ E.T)/temp;  same[i,j] = (label_i == label_j)
  S_j   = sum_i exp(logits[i,j]) * (1 - same[i,j])     (masked column sums)
  ce_ij = log(exp(-eps) + S_j * exp(-logits[i,j]))     for same-label i != j
  loss  = sum_j (1/count_j) * sum_i ce_ij / B

Columns are sharded over 8 cores after a host-side label sort, so core c's
1024 columns have all their same-label rows in one contiguous "window" of
the sorted row order.  Everything runs in bf16 (tolerance 2e-2; bf16
perturbs logits/temp by ~4e-3).

Per core the 8192 rows split two ways, each row counted exactly once:

  column region (2560 rows = window 1536 + mainB 1024, PSUM banks 0-4):
    5 bf16 matmuls per col-tile, a one-hot mask matmul adds -4.5/temp to
    the tile's 448-row label slice, then ONE ACT exp over all 2560 rows
    with fused column accum -> the non-transposed part of S_j.  The exp
    dump is fp32; its slice range doubles as P for the numerator (masked
    same-label entries vanish from the accum, and 1/P recovers them for
    the Ln path).

  transposed rows (5632 = 44 blocks of 128, PSUM banks 6/7): bf16 matmul
    [128 rows x 512 cols] -> DVE tensor_scalar affine to int16 whose bit
    pattern IS bf16 exp(l/temp) (Schraudolph; ±2% sawtooth that averages
    out) -> PE ones-matmul sums over rows into two [1,512] PSUM
    accumulators living ALONE in bank 5 (a start=True matmul clears
    has_written for its whole bank, so nothing else may share it).  Every
    4th block runs its exp on ACT (exact) instead of DVE to balance load.

Numerator per tile: R = 1/P (DVE), then ACT Ln(m_j*R + 1) with fused accum
= sum of (ce+eps) over the tile's same-label rows; m_j = S_j*e^(eps-C).
Host subtracts eps*count_j and the diagonal's self-term.
"""
import numpy as np
import ml_dtypes
from contextlib import ExitStack

import concourse.bacc as bacc
import concourse.tile as tile
from concourse import mybir
from concourse.bass_utils import run_bass_kernel_spmd

B, D = 8192, 128
NCLS = 100
NCORES = 8
COLS = B // NCORES            # 1024 columns per core
NCT = COLS // 128             # 8 col-tiles per core
W = 1536                      # window rows (span + pad)
SW = 448                      # per-tile numerator slice width
YROWS = 1024                  # column-layout main rows
CREG = W + YROWS              # 2560-word column PSUM region
TR = B - CREG                 # 5632 transposed rows
NBLK = TR // 128              # 44 transposed blocks
ACT_BLK_EVERY = 4             # every 4th trans block exps on ACT not DVE

TEMP = 0.07
EPS = 0.25
SCALE = float(np.float32(1.0) / np.float32(TEMP))
MASKVAL = -4.5                                      # bf16-exact additive mask
C_USED = 4.5 * SCALE                                # mask size in logit units
MCONST = float(np.exp(EPS - C_USED))                # e^(eps-C)
LOG2E = 1.4426950408889634
K2 = 128.0 * LOG2E * SCALE                          # Schraudolph int16/bf16
B2 = 128.0 * 127.0 - 7.42

# PSUM word layout (4096 fp32/partition, bank = 512 words).
PC = 0                        # column region            [0, 2560)
PS = 2560                     # S halves (bank 5)        [2560, 3072)
PT0 = 3072                    # trans chunk buf 0        [3072, 3584)
PT1 = 3584                    # trans chunk buf 1        [3584, 4096)

_cache = {}


def _patch_act_tables():
    """Steer Exp and Ln onto the one table set holding both, so Exp/Ln
    alternation doesn't thrash ACT_TABLE_LOADs."""
    import concourse.hw_specs as hw_specs
    from concourse import mybir as _mb
    if getattr(bacc, "_act_tables_patched", False):
        return
    orig = hw_specs.get_activation_tables

    def steer(arch):
        t = orig(arch)
        exp, ln = (_mb.ActivationFunctionType.Exp, _mb.ActivationFunctionType.Ln)
        if "natural_log_exp_and_others" not in t:
            return t
        return {k: (fns if k == "natural_log_exp_and_others"
                    else fns - {exp, ln}) for k, fns in t.items()}

    bacc.get_activation_tables = steer
    bacc._act_tables_patched = True


def _bank_split(r0, r1):
    out = []
    while r0 < r1:
        nxt = min(r1, (r0 // 512 + 1) * 512)
        out.append((r0, nxt))
        r0 = nxt
    return out


def _build(los):
    dt = mybir.dt
    _patch_act_tables()
    nc = bacc.Bacc("TRN2", target_bir_lowering=False, debug=False,
                   num_devices=NCORES)
    et_win = nc.dram_tensor("et_win", [D, W], dt.bfloat16,
                            kind="ExternalInput").ap()
    et_y = nc.dram_tensor("et_y", [D, YROWS], dt.bfloat16,
                          kind="ExternalInput").ap()
    et_tr = nc.dram_tensor("et_tr", [D, TR], dt.bfloat16,
                           kind="ExternalInput").ap()
    et_own = nc.dram_tensor("et_own", [D, COLS], dt.bfloat16,
                            kind="ExternalInput").ap()
    oh_win = nc.dram_tensor("oh_win", [NCLS, W], dt.bfloat16,
                            kind="ExternalInput").ap()
    ohn_own = nc.dram_tensor("ohn_own", [NCLS, COLS], dt.bfloat16,
                             kind="ExternalInput").ap()
    out = nc.dram_tensor("out", [128, 2 * NCT], dt.float32,
                         kind="ExternalOutput").ap()
    scratch = nc.dram_tensor("scratch", [1, COLS], dt.float32,
                             kind="Internal").ap()

    with tile.TileContext(nc) as tc:
        with ExitStack() as ctx:
            cpool = ctx.enter_context(tc.tile_pool(name="consts", bufs=1))
            dpool = ctx.enter_context(tc.tile_pool(name="dumps", bufs=2))
            vpool = ctx.enter_context(tc.tile_pool(name="conv", bufs=2))
            ps_pool = ctx.enter_context(
                tc.tile_pool(name="psum", bufs=1, space="PSUM"))

            # tiny warm-up activation so the Exp/Ln ACT table loads during
            # the input-DMA dead time
            warm = cpool.tile([128, 1], dt.float32)
            nc.vector.memset(warm[:], 0.0)
            nc.scalar.activation(warm[:], warm[:],
                                 mybir.ActivationFunctionType.Exp)

            t_win = cpool.tile([D, W], dt.bfloat16)
            t_y = cpool.tile([D, YROWS], dt.bfloat16)
            t_tr = cpool.tile([D, TR], dt.bfloat16)
            t_own = cpool.tile([D, COLS], dt.bfloat16)
            t_oh = cpool.tile([NCLS, W], dt.bfloat16)
            t_ohn = cpool.tile([NCLS, COLS], dt.bfloat16)
            # first col-tile needs et_own[:, :128] + et_win: split those DMAs
            # so the first matmuls start as early as possible
            nc.sync.dma_start(t_own[:, 0:256], et_own[:, 0:256])
            nc.sync.dma_start(t_win[:, 0:512], et_win[:, 0:512])
            nc.sync.dma_start(t_win[:, 512:], et_win[:, 512:])
            nc.sync.dma_start(t_own[:, 256:], et_own[:, 256:])
            nc.scalar.dma_start(t_y[:], et_y[:])
            nc.gpsimd.dma_start(t_ohn[:], ohn_own[:])
            nc.gpsimd.dma_start(t_oh[:], oh_win[:])
            for q in range(4):
                eng = nc.scalar if q % 2 else nc.sync
                eng.dma_start(t_tr[:, q * (TR // 4):(q + 1) * (TR // 4)],
                              et_tr[:, q * (TR // 4):(q + 1) * (TR // 4)])

            ones = cpool.tile([128, 1], dt.bfloat16)
            nc.vector.memset(ones[:], 1.0)
            s_part = cpool.tile([128, NCT], dt.float32)
            scat_sb = cpool.tile([128, NCT], dt.float32)
            a_part = cpool.tile([128, NCT], dt.float32)
            s_sb = cpool.tile([128, NCT], dt.float32)
            m_sb = cpool.tile([128, NCT], dt.float32)
            rbig = cpool.tile([128, NCT, SW], dt.float32)
            ssb = cpool.tile([128, 512], dt.float32)

            big = ps_pool.tile([128, 4096], dt.float32)
            s_half = [big[0:1, PS:PS + 512], big[32:33, PS:PS + 512]]

            def trans_block(blk):
                lhs = t_tr[:, blk * 128:(blk + 1) * 128]
                on_act = (blk % ACT_BLK_EVERY) == (ACT_BLK_EVERY - 1)
                for h, pt in ((0, PT0), (1, PT1)):
                    tbuf = big[:, pt:pt + 512]
                    nc.tensor.matmul(tbuf, lhs,
                                     t_own[:, h * 512:(h + 1) * 512],
                                     start=True, stop=True,
                                     skip_group_check=True)
                    if on_act:
                        cb = vpool.tile([128, 512], dt.bfloat16,
                                        tag=f"ab{h}")
                        nc.scalar.activation(
                            cb[:], tbuf, mybir.ActivationFunctionType.Exp,
                            scale=SCALE)
                        rd = cb[:]
                    else:
                        cb = vpool.tile([128, 512], dt.int16, tag=f"cv{h}")
                        nc.vector.tensor_scalar(
                            out=cb[:], in0=tbuf, scalar1=K2, scalar2=B2,
                            op0=mybir.AluOpType.mult,
                            op1=mybir.AluOpType.add)
                        rd = cb[:].bitcast(dt.bfloat16)
                    nc.tensor.matmul(
                        s_half[h], ones[:], rd,
                        start=(blk == 0), stop=(blk == NBLK - 1),
                        skip_group_check=True)

            def emit_mln(ct):
                nc.vector.tensor_add(s_sb[:, ct:ct + 1],
                                     s_part[:, ct:ct + 1],
                                     scat_sb[:, ct:ct + 1])
                nc.vector.tensor_scalar_mul(
                    m_sb[:, ct:ct + 1], s_sb[:, ct:ct + 1], MCONST)
                ld = dpool.tile([128, SW], dt.bfloat16, tag="lndump")
                nc.scalar.activation(
                    ld[:], rbig[:, ct, :], mybir.ActivationFunctionType.Ln,
                    scale=m_sb[:, ct:ct + 1], bias=1.0,
                    accum_out=a_part[:, ct:ct + 1])

            blk_sched = [8, 8, 8, 8, 8, 4, 0, 0]
            blk_iter = iter(range(NBLK))

            for ct in range(NCT):
                lo = los[ct]
                lhs_et = t_own[:, ct * 128:(ct + 1) * 128]
                lhs_oh = t_ohn[:, ct * 128:(ct + 1) * 128]

                # ---- column region: window then mainB ----
                for r0, r1 in ((0, 512), (512, 1024), (1024, 1536)):
                    nc.tensor.matmul(big[:, PC + r0:PC + r1], lhs_et,
                                     t_win[:, r0:r1], start=True, stop=False,
                                     skip_group_check=True)
                for r0, r1 in _bank_split(lo, lo + SW):
                    nc.tensor.matmul(big[:, PC + r0:PC + r1], lhs_oh,
                                     t_oh[:, r0:r1], start=False, stop=True,
                                     skip_group_check=True)
                for r0, r1 in ((0, 512), (512, YROWS)):
                    nc.tensor.matmul(big[:, PC + W + r0:PC + W + r1], lhs_et,
                                     t_y[:, r0:r1], start=True, stop=True,
                                     skip_group_check=True)

                # one exp over the whole region; fp32 dump doubles as P
                xyd = dpool.tile([128, CREG], dt.float32, tag="xydump")
                nc.scalar.activation(
                    xyd[:], big[:, PC:PC + CREG],
                    mybir.ActivationFunctionType.Exp, scale=SCALE,
                    accum_out=s_part[:, ct:ct + 1])
                nc.vector.reciprocal_approx_fast(
                    out=rbig[:, ct, :], in_=xyd[:, lo:lo + SW])

                for _ in range(blk_sched[ct]):
                    trans_block(next(blk_iter))

                if ct == 5:
                    # S halves -> scratch DRAM -> [128, 8] scatter
                    nc.vector.tensor_copy(ssb[:], big[:, PS:PS + 512])
                    nc.sync.dma_start(
                        scratch[:].rearrange("o (ph r) -> ph (o r)", ph=2),
                        ssb[:].rearrange("(a b) r -> a b r", b=32)[0:2, 0:1])
                    nc.sync.dma_start(
                        scat_sb[:],
                        scratch[:].rearrange("o (c p) -> (o p) c", p=128))
                if ct == 6:
                    for c2 in range(6):
                        emit_mln(c2)
            emit_mln(6)
            emit_mln(7)

            nc.sync.dma_start(out[:, 0:NCT], a_part[:])
            nc.sync.dma_start(out[:, NCT:], s_sb[:])
    nc.compile()
    return nc


def _get_nc(los):
    key = ("v3", tuple(los))
    if key not in _cache:
        _cache[key] = _build(tuple(los))
    return _cache[key]


def _prepare(embeds, labels):
    embeds = np.ascontiguousarray(np.asarray(embeds, dtype=np.float32))
    labels_i = np.asarray(labels).astype(np.int64)
    assert embeds.shape == (B, D)

    perm = np.argsort(labels_i, kind="stable")
    lab = labels_i[perm]
    emb = embeds[perm]
    ebf = emb.astype(ml_dtypes.bfloat16)
    et = np.ascontiguousarray(ebf.T)                      # [D, B] bf16
    oh = np.zeros((NCLS, B), dtype=ml_dtypes.bfloat16)
    oh[lab, np.arange(B)] = ml_dtypes.bfloat16(1.0)
    ohn = oh * ml_dtypes.bfloat16(MASKVAL)

    starts = np.searchsorted(lab, np.arange(NCLS), side="left")
    ends = np.searchsorted(lab, np.arange(NCLS), side="right")

    s_min = [10**9] * NCT
    e_max = [0] * NCT
    spans = []
    for c in range(NCORES):
        lo, hi = c * COLS, (c + 1) * COLS
        r_lo = int(starts[lab[lo]])
        r_hi = int(ends[lab[hi - 1]])
        spans.append((r_lo, r_hi))
        assert r_hi - r_lo <= W, f"span overflow: {r_hi - r_lo}"
        for ct in range(NCT):
            cl, ch = lo + ct * 128, lo + (ct + 1) * 128
            s_ct = int(starts[lab[cl]]) - r_lo
            e_ct = int(ends[lab[ch - 1]]) - r_lo
            s_min[ct] = min(s_min[ct], s_ct)
            e_max[ct] = max(e_max[ct], e_ct)
    los = []
    for ct in range(NCT):
        lo_ct = max(0, min(e_max[ct] - SW, s_min[ct], W - SW))
        assert lo_ct <= s_min[ct] and e_max[ct] <= lo_ct + SW, (
            f"slice infeasible ct={ct}: [{s_min[ct]},{e_max[ct]}] "
            f"vs lo={lo_ct} SW={SW}")
        los.append(lo_ct)

    in_maps = []
    for c in range(NCORES):
        r_lo, r_hi = spans[c]
        span = r_hi - r_lo
        fill = W - span
        after = np.arange(r_hi, min(B, r_hi + fill))
        need = fill - len(after)
        before = np.arange(r_lo - need, r_lo) if need > 0 else np.arange(0)
        win_rows = np.concatenate([np.arange(r_lo, r_hi), after, before])
        assert len(win_rows) == W
        main_mask = np.ones(B, dtype=bool)
        main_mask[win_rows] = False
        main_idx = np.nonzero(main_mask)[0]
        assert len(main_idx) == YROWS + TR
        lo = c * COLS
        in_maps.append({
            "et_win": np.ascontiguousarray(et[:, win_rows]),
            "et_y": np.ascontiguousarray(et[:, main_idx[:YROWS]]),
            "et_tr": np.ascontiguousarray(et[:, main_idx[YROWS:]]),
            "et_own": np.ascontiguousarray(et[:, lo:lo + COLS]),
            "oh_win": np.ascontiguousarray(oh[:, win_rows]),
            "ohn_own": np.ascontiguousarray(ohn[:, lo:lo + COLS]),
        })
    return in_maps, los, lab, ebf


def _combine(results, lab, ebf):
    S = np.empty(B, dtype=np.float64)
    A = np.empty(B, dtype=np.float64)
    for c in range(NCORES):
        o = results[c]["out"].astype(np.float64)
        A[c * COLS:(c + 1) * COLS] = o[:, 0:NCT].T.reshape(-1)
        S[c * COLS:(c + 1) * COLS] = o[:, NCT:].T.reshape(-1)

    counts = np.bincount(lab, minlength=NCLS)
    count_j = counts[lab].astype(np.float64) - 1.0
    l_jj = (ebf.astype(np.float64) ** 2).sum(1) * SCALE
    u_jj = np.log1p(S * np.exp(EPS - l_jj))
    numer = A - EPS * count_j - u_jj
    loss = (numer / count_j).sum() / B
    return np.asarray(loss, dtype=np.float32)


def kernel(embeds, labels):
    in_maps, los, lab, ebf = _prepare(embeds, labels)
    nc = _get_nc(los)
    res = run_bass_kernel_spmd(nc, in_maps, list(range(NCORES)))
    return _combine(res.results, lab, ebf)


# revision 14
# speedup vs baseline: 1.0404x; 1.0404x over previous
"""EpsSupInfoNCE loss on 8 Trainium2 NeuronCores — v3.

Math (reference): logits = (E @ E.T)/temp;  same[i,j] = (label_i == label_j)
  S_j   = sum_i exp(logits[i,j]) * (1 - same[i,j])     (masked column sums)
  ce_ij = log(exp(-eps) + S_j * exp(-logits[i,j]))     for same-label i != j
  loss  = sum_j (1/count_j) * sum_i ce_ij / B

Columns are sharded over 8 cores after a host-side label sort, so core c's
1024 columns have all their same-label rows in one contiguous "window" of
the sorted row order.  Everything runs in bf16 (tolerance 2e-2; bf16
perturbs logits/temp by ~4e-3).

Per core the 8192 rows split two ways, each row counted exactly once:

  column region (2560 rows = window 1536 + mainB 1024, PSUM banks 0-4):
    5 bf16 matmuls per col-tile, a one-hot mask matmul adds -4.5/temp to
    the tile's 448-row label slice, then ONE ACT exp over all 2560 rows
    with fused column accum -> the non-transposed part of S_j.  The exp
    dump is fp32; its slice range doubles as P for the numerator (masked
    same-label entries vanish from the accum, and 1/P recovers them for
    the Ln path).

  transposed rows (5632 = 44 blocks of 128, PSUM banks 6/7): bf16 matmul
    [128 rows x 512 cols] -> DVE tensor_scalar affine to int16 whose bit
    pattern IS bf16 exp(l/temp) (Schraudolph; ±2% sawtooth that averages
    out) -> PE ones-matmul sums over rows into two [1,512] PSUM
    accumulators living ALONE in bank 5 (a start=True matmul clears
    has_written for its whole bank, so nothing else may share it).  Every
    4th block runs its exp on ACT (exact) instead of DVE to balance load.

Numerator per tile: R = 1/P (DVE), then ACT Ln(m_j*R + 1) with fused accum
= sum of (ce+eps) over the tile's same-label rows; m_j = S_j*e^(eps-C).
Host subtracts eps*count_j and the diagonal's self-term.
"""
import numpy as np
import ml_dtypes
from contextlib import ExitStack

import concourse.bacc as bacc
import concourse.tile as tile
from concourse import mybir
from concourse.bass_utils import run_bass_kernel_spmd

B, D = 8192, 128
NCLS = 100
NCORES = 8
COLS = B // NCORES            # 1024 columns per core
NCT = COLS // 128             # 8 col-tiles per core
W = 1536                      # window rows (span + pad)
SW = 448                      # per-tile numerator slice width
YROWS = 1024                  # column-layout main rows
CREG = W + YROWS              # 2560-word column PSUM region
TR = B - CREG                 # 5632 transposed rows
NBLK = TR // 128              # 44 transposed blocks
ACT_BLK_EVERY = 4             # every 4th trans block exps on ACT not DVE

TEMP = 0.07
EPS = 0.25
SCALE = float(np.float32(1.0) / np.float32(TEMP))
MASKVAL = -4.5                                      # bf16-exact additive mask
C_USED = 4.5 * SCALE                                # mask size in logit units
MCONST = float(np.exp(EPS - C_USED))                # e^(eps-C)
LOG2E = 1.4426950408889634
K2 = 128.0 * LOG2E * SCALE                          # Schraudolph int16/bf16
B2 = 128.0 * 127.0 - 7.42

# PSUM word layout (4096 fp32/partition, bank = 512 words).
PC = 0                        # column region            [0, 2560)
PS = 2560                     # S halves (bank 5)        [2560, 3072)
PT0 = 3072                    # trans chunk buf 0        [3072, 3584)
PT1 = 3584                    # trans chunk buf 1        [3584, 4096)

_cache = {}


def _patch_act_tables():
    """Steer Exp and Ln onto the one table set holding both, so Exp/Ln
    alternation doesn't thrash ACT_TABLE_LOADs."""
    import concourse.hw_specs as hw_specs
    from concourse import mybir as _mb
    if getattr(bacc, "_act_tables_patched", False):
        return
    orig = hw_specs.get_activation_tables

    def steer(arch):
        t = orig(arch)
        exp, ln = (_mb.ActivationFunctionType.Exp, _mb.ActivationFunctionType.Ln)
        if "natural_log_exp_and_others" not in t:
            return t
        return {k: (fns if k == "natural_log_exp_and_others"
                    else fns - {exp, ln}) for k, fns in t.items()}

    bacc.get_activation_tables = steer
    bacc._act_tables_patched = True


def _bank_split(r0, r1):
    out = []
    while r0 < r1:
        nxt = min(r1, (r0 // 512 + 1) * 512)
        out.append((r0, nxt))
        r0 = nxt
    return out


def _build(los):
    dt = mybir.dt
    _patch_act_tables()
    nc = bacc.Bacc("TRN2", target_bir_lowering=False, debug=False,
                   num_devices=NCORES)
    et_win = nc.dram_tensor("et_win", [D, W], dt.bfloat16,
                            kind="ExternalInput").ap()
    et_y = nc.dram_tensor("et_y", [D, YROWS], dt.bfloat16,
                          kind="ExternalInput").ap()
    et_tr = nc.dram_tensor("et_tr", [D, TR], dt.bfloat16,
                           kind="ExternalInput").ap()
    et_own = nc.dram_tensor("et_own", [D, COLS], dt.bfloat16,
                            kind="ExternalInput").ap()
    oh_win = nc.dram_tensor("oh_win", [NCLS, W], dt.bfloat16,
                            kind="ExternalInput").ap()
    ohn_own = nc.dram_tensor("ohn_own", [NCLS, COLS], dt.bfloat16,
                             kind="ExternalInput").ap()
    out = nc.dram_tensor("out", [128, 2 * NCT], dt.float32,
                         kind="ExternalOutput").ap()
    scratch = nc.dram_tensor("scratch", [1, COLS], dt.float32,
                             kind="Internal").ap()

    with tile.TileContext(nc) as tc:
        with ExitStack() as ctx:
            cpool = ctx.enter_context(tc.tile_pool(name="consts", bufs=1))
            dpool = ctx.enter_context(tc.tile_pool(name="dumps", bufs=2))
            vpool = ctx.enter_context(tc.tile_pool(name="conv", bufs=2))
            ps_pool = ctx.enter_context(
                tc.tile_pool(name="psum", bufs=1, space="PSUM"))

            # tiny warm-up activation so the Exp+Ln ACT table set loads
            # during the input-DMA dead time (Ln forces natural_log_exp set)
            warm = cpool.tile([128, 1], dt.float32)
            nc.vector.memset(warm[:], 1.0)
            nc.scalar.activation(warm[:], warm[:],
                                 mybir.ActivationFunctionType.Ln)

            t_win = cpool.tile([D, W], dt.bfloat16)
            t_y = cpool.tile([D, YROWS], dt.bfloat16)
            t_tr = cpool.tile([D, TR], dt.bfloat16)
            t_own = cpool.tile([D, COLS], dt.bfloat16)
            t_oh = cpool.tile([NCLS, W], dt.bfloat16)
            t_ohn = cpool.tile([NCLS, COLS], dt.bfloat16)
            # first col-tile needs et_own[:, :128] + et_win: split those DMAs
            # so the first matmuls start as early as possible
            nc.sync.dma_start(t_own[:, 0:256], et_own[:, 0:256])
            nc.sync.dma_start(t_win[:, 0:512], et_win[:, 0:512])
            nc.sync.dma_start(t_win[:, 512:], et_win[:, 512:])
            nc.sync.dma_start(t_own[:, 256:], et_own[:, 256:])
            nc.gpsimd.dma_start(t_ohn[:], ohn_own[:])
            nc.gpsimd.dma_start(t_oh[:], oh_win[:])
            nc.sync.dma_start(t_y[:], et_y[:])
            for q in range(4):
                eng = nc.gpsimd if q >= 2 else nc.sync
                eng.dma_start(t_tr[:, q * (TR // 4):(q + 1) * (TR // 4)],
                              et_tr[:, q * (TR // 4):(q + 1) * (TR // 4)])

            ones = cpool.tile([128, 1], dt.bfloat16)
            nc.vector.memset(ones[:], 1.0)
            s_part = cpool.tile([128, NCT], dt.float32)
            scat_sb = cpool.tile([128, NCT], dt.float32)
            a_part = cpool.tile([128, NCT], dt.float32)
            s_sb = cpool.tile([128, NCT], dt.float32)
            m_sb = cpool.tile([128, NCT], dt.float32)
            rbig = cpool.tile([128, NCT, SW], dt.float32)
            ssb = cpool.tile([128, 512], dt.float32)

            big = ps_pool.tile([128, 4096], dt.float32)
            s_half = [big[0:1, PS:PS + 512], big[32:33, PS:PS + 512]]

            # The ones-matmuls for block k are emitted during block k+1, so
            # the PE FIFO never stalls waiting for block k's DVE convert.
            pending = []          # [(blk, rd0, rd1)]

            def emit_ones(blk, rd0, rd1):
                for h, rd in ((0, rd0), (1, rd1)):
                    nc.tensor.matmul(
                        s_half[h], ones[:], rd,
                        start=(blk == 0), stop=(blk == NBLK - 1),
                        skip_group_check=True)

            def trans_block(blk):
                lhs = t_tr[:, blk * 128:(blk + 1) * 128]
                on_act = (blk % ACT_BLK_EVERY) == (ACT_BLK_EVERY - 1)
                bufs = []
                for h, pt in ((0, PT0), (1, PT1)):
                    nc.tensor.matmul(big[:, pt:pt + 512], lhs,
                                     t_own[:, h * 512:(h + 1) * 512],
                                     start=True, stop=True,
                                     skip_group_check=True)
                if pending:
                    emit_ones(*pending.pop())
                for h, pt in ((0, PT0), (1, PT1)):
                    tbuf = big[:, pt:pt + 512]
                    if on_act:
                        cb = vpool.tile([128, 512], dt.bfloat16,
                                        tag=f"ab{h}")
                        nc.scalar.activation(
                            cb[:], tbuf, mybir.ActivationFunctionType.Exp,
                            scale=SCALE)
                        bufs.append(cb[:])
                    else:
                        cb = vpool.tile([128, 512], dt.int16, tag=f"cv{h}")
                        nc.vector.tensor_scalar(
                            out=cb[:], in0=tbuf, scalar1=K2, scalar2=B2,
                            op0=mybir.AluOpType.mult,
                            op1=mybir.AluOpType.add)
                        bufs.append(cb[:].bitcast(dt.bfloat16))
                pending.append((blk, bufs[0], bufs[1]))

            def emit_mln(ct):
                nc.vector.tensor_add(s_sb[:, ct:ct + 1],
                                     s_part[:, ct:ct + 1],
                                     scat_sb[:, ct:ct + 1])
                nc.vector.tensor_scalar_mul(
                    m_sb[:, ct:ct + 1], s_sb[:, ct:ct + 1], MCONST)
                ld = dpool.tile([128, SW], dt.bfloat16, tag="lndump")
                nc.scalar.activation(
                    ld[:], rbig[:, ct, :], mybir.ActivationFunctionType.Ln,
                    scale=m_sb[:, ct:ct + 1], bias=1.0,
                    accum_out=a_part[:, ct:ct + 1])

            blk_sched = [8, 8, 8, 8, 8, 4, 0, 0]
            blk_iter = iter(range(NBLK))

            for ct in range(NCT):
                lo = los[ct]
                lhs_et = t_own[:, ct * 128:(ct + 1) * 128]
                lhs_oh = t_ohn[:, ct * 128:(ct + 1) * 128]

                # ---- column region: window then mainB ----
                for r0, r1 in ((0, 512), (512, 1024), (1024, 1536)):
                    nc.tensor.matmul(big[:, PC + r0:PC + r1], lhs_et,
                                     t_win[:, r0:r1], start=True, stop=False,
                                     skip_group_check=True)
                for r0, r1 in _bank_split(lo, lo + SW):
                    nc.tensor.matmul(big[:, PC + r0:PC + r1], lhs_oh,
                                     t_oh[:, r0:r1], start=False, stop=True,
                                     skip_group_check=True)
                for r0, r1 in ((0, 512), (512, YROWS)):
                    nc.tensor.matmul(big[:, PC + W + r0:PC + W + r1], lhs_et,
                                     t_y[:, r0:r1], start=True, stop=True,
                                     skip_group_check=True)

                # one exp over the whole region; fp32 dump doubles as P
                xyd = dpool.tile([128, CREG], dt.float32, tag="xydump")
                nc.scalar.activation(
                    xyd[:], big[:, PC:PC + CREG],
                    mybir.ActivationFunctionType.Exp, scale=SCALE,
                    accum_out=s_part[:, ct:ct + 1])
                nc.vector.reciprocal_approx_fast(
                    out=rbig[:, ct, :], in_=xyd[:, lo:lo + SW])

                for _ in range(blk_sched[ct]):
                    trans_block(next(blk_iter))

                if ct == 5:
                    # S halves -> scratch DRAM -> [128, 8] scatter
                    nc.vector.tensor_copy(ssb[:], big[:, PS:PS + 512])
                    nc.sync.dma_start(
                        scratch[:].rearrange("o (ph r) -> ph (o r)", ph=2),
                        ssb[:].rearrange("(a b) r -> a b r", b=32)[0:2, 0:1])
                    nc.sync.dma_start(
                        scat_sb[:],
                        scratch[:].rearrange("o (c p) -> (o p) c", p=128))
                if ct == 6:
                    for c2 in range(6):
                        emit_mln(c2)
            emit_mln(6)
            emit_mln(7)

            nc.sync.dma_start(out[:, 0:NCT], a_part[:])
            nc.sync.dma_start(out[:, NCT:], s_sb[:])
    nc.compile()
    return nc


def _get_nc(los):
    key = ("v3", tuple(los))
    if key not in _cache:
        _cache[key] = _build(tuple(los))
    return _cache[key]


def _prepare(embeds, labels):
    embeds = np.ascontiguousarray(np.asarray(embeds, dtype=np.float32))
    labels_i = np.asarray(labels).astype(np.int64)
    assert embeds.shape == (B, D)

    perm = np.argsort(labels_i, kind="stable")
    lab = labels_i[perm]
    emb = embeds[perm]
    ebf = emb.astype(ml_dtypes.bfloat16)
    et = np.ascontiguousarray(ebf.T)                      # [D, B] bf16
    oh = np.zeros((NCLS, B), dtype=ml_dtypes.bfloat16)
    oh[lab, np.arange(B)] = ml_dtypes.bfloat16(1.0)
    ohn = oh * ml_dtypes.bfloat16(MASKVAL)

    starts = np.searchsorted(lab, np.arange(NCLS), side="left")
    ends = np.searchsorted(lab, np.arange(NCLS), side="right")

    s_min = [10**9] * NCT
    e_max = [0] * NCT
    spans = []
    for c in range(NCORES):
        lo, hi = c * COLS, (c + 1) * COLS
        r_lo = int(starts[lab[lo]])
        r_hi = int(ends[lab[hi - 1]])
        spans.append((r_lo, r_hi))
        assert r_hi - r_lo <= W, f"span overflow: {r_hi - r_lo}"
        for ct in range(NCT):
            cl, ch = lo + ct * 128, lo + (ct + 1) * 128
            s_ct = int(starts[lab[cl]]) - r_lo
            e_ct = int(ends[lab[ch - 1]]) - r_lo
            s_min[ct] = min(s_min[ct], s_ct)
            e_max[ct] = max(e_max[ct], e_ct)
    los = []
    for ct in range(NCT):
        lo_ct = max(0, min(e_max[ct] - SW, s_min[ct], W - SW))
        assert lo_ct <= s_min[ct] and e_max[ct] <= lo_ct + SW, (
            f"slice infeasible ct={ct}: [{s_min[ct]},{e_max[ct]}] "
            f"vs lo={lo_ct} SW={SW}")
        los.append(lo_ct)

    in_maps = []
    for c in range(NCORES):
        r_lo, r_hi = spans[c]
        span = r_hi - r_lo
        fill = W - span
        after = np.arange(r_hi, min(B, r_hi + fill))
        need = fill - len(after)
        before = np.arange(r_lo - need, r_lo) if need > 0 else np.arange(0)
        win_rows = np.concatenate([np.arange(r_lo, r_hi), after, before])
        assert len(win_rows) == W
        main_mask = np.ones(B, dtype=bool)
        main_mask[win_rows] = False
        main_idx = np.nonzero(main_mask)[0]
        assert len(main_idx) == YROWS + TR
        lo = c * COLS
        in_maps.append({
            "et_win": np.ascontiguousarray(et[:, win_rows]),
            "et_y": np.ascontiguousarray(et[:, main_idx[:YROWS]]),
            "et_tr": np.ascontiguousarray(et[:, main_idx[YROWS:]]),
            "et_own": np.ascontiguousarray(et[:, lo:lo + COLS]),
            "oh_win": np.ascontiguousarray(oh[:, win_rows]),
            "ohn_own": np.ascontiguousarray(ohn[:, lo:lo + COLS]),
        })
    return in_maps, los, lab, ebf


def _combine(results, lab, ebf):
    S = np.empty(B, dtype=np.float64)
    A = np.empty(B, dtype=np.float64)
    for c in range(NCORES):
        o = results[c]["out"].astype(np.float64)
        A[c * COLS:(c + 1) * COLS] = o[:, 0:NCT].T.reshape(-1)
        S[c * COLS:(c + 1) * COLS] = o[:, NCT:].T.reshape(-1)

    counts = np.bincount(lab, minlength=NCLS)
    count_j = counts[lab].astype(np.float64) - 1.0
    l_jj = (ebf.astype(np.float64) ** 2).sum(1) * SCALE
    u_jj = np.log1p(S * np.exp(EPS - l_jj))
    numer = A - EPS * count_j - u_jj
    loss = (numer / count_j).sum() / B
    return np.asarray(loss, dtype=np.float32)


def kernel(embeds, labels):
    in_maps, los, lab, ebf = _prepare(embeds, labels)
    nc = _get_nc(los)
    res = run_bass_kernel_spmd(nc, in_maps, list(range(NCORES)))
    return _combine(res.results, lab, ebf)


# revision 20
# speedup vs baseline: 1.1066x; 1.0636x over previous
"""EpsSupInfoNCE loss on 8 Trainium2 NeuronCores — v3.

Math (reference): logits = (E @ E.T)/temp;  same[i,j] = (label_i == label_j)
  S_j   = sum_i exp(logits[i,j]) * (1 - same[i,j])     (masked column sums)
  ce_ij = log(exp(-eps) + S_j * exp(-logits[i,j]))     for same-label i != j
  loss  = sum_j (1/count_j) * sum_i ce_ij / B

Columns are sharded over 8 cores after a host-side label sort, so core c's
1024 columns have all their same-label rows in one contiguous "window" of
the sorted row order.  Everything runs in bf16 (tolerance 2e-2; bf16
perturbs logits/temp by ~4e-3).

Per core the 8192 rows split two ways, each row counted exactly once:

  column region (2560 rows = window 1536 + mainB 1024, PSUM banks 0-4):
    5 bf16 matmuls per col-tile, a one-hot mask matmul adds -4.5/temp to
    the tile's 448-row label slice, then ONE ACT exp over all 2560 rows
    with fused column accum -> the non-transposed part of S_j.  The exp
    dump is fp32; its slice range doubles as P for the numerator (masked
    same-label entries vanish from the accum, and 1/P recovers them for
    the Ln path).

  transposed rows (5632 = 44 blocks of 128, PSUM banks 6/7): bf16 matmul
    [128 rows x 512 cols] -> DVE tensor_scalar affine to int16 whose bit
    pattern IS bf16 exp(l/temp) (Schraudolph; ±2% sawtooth that averages
    out) -> PE ones-matmul sums over rows into two [1,512] PSUM
    accumulators living ALONE in bank 5 (a start=True matmul clears
    has_written for its whole bank, so nothing else may share it).  Every
    4th block runs its exp on ACT (exact) instead of DVE to balance load.

Numerator per tile: R = 1/P (DVE), then ACT Ln(m_j*R + 1) with fused accum
= sum of (ce+eps) over the tile's same-label rows; m_j = S_j*e^(eps-C).
Host subtracts eps*count_j and the diagonal's self-term.
"""
import numpy as np
import ml_dtypes
from contextlib import ExitStack

import concourse.bacc as bacc
import concourse.tile as tile
from concourse import mybir
from concourse.bass_utils import run_bass_kernel_spmd

B, D = 8192, 128
NCLS = 100
NCORES = 8
COLS = B // NCORES            # 1024 columns per core
NCT = COLS // 128             # 8 col-tiles per core
W = 1280                      # window rows (span + pad) = column group A
SW = 448                      # per-tile numerator slice width
YROWS = 1280                  # column-layout main rows  = column group B
CREG = W + YROWS              # 2560-word column PSUM region
TR = B - CREG                 # 5632 transposed rows
NBLK = TR // 128              # 44 transposed blocks
ACT_BLK_EVERY = 4             # every 4th trans block exps on ACT not DVE

TEMP = 0.07
EPS = 0.25
SCALE = float(np.float32(1.0) / np.float32(TEMP))
MASKVAL = -4.5                                      # bf16-exact additive mask
C_USED = 4.5 * SCALE                                # mask size in logit units
MCONST = float(np.exp(EPS - C_USED))                # e^(eps-C)
LOG2E = 1.4426950408889634
K2 = 128.0 * LOG2E * SCALE                          # Schraudolph int16/bf16
B2 = 128.0 * 127.0 - 7.42

# PSUM word layout (4096 fp32/partition, bank = 512 words).
PC = 0                        # column region            [0, 2560)
PS = 2560                     # S halves (bank 5)        [2560, 3072)
PT0 = 3072                    # trans chunk buf 0        [3072, 3584)
PT1 = 3584                    # trans chunk buf 1        [3584, 4096)

_cache = {}


def _patch_act_tables():
    """Steer Exp and Ln onto the one table set holding both, so Exp/Ln
    alternation doesn't thrash ACT_TABLE_LOADs."""
    import concourse.hw_specs as hw_specs
    from concourse import mybir as _mb
    if getattr(bacc, "_act_tables_patched", False):
        return
    orig = hw_specs.get_activation_tables

    def steer(arch):
        t = orig(arch)
        exp, ln = (_mb.ActivationFunctionType.Exp, _mb.ActivationFunctionType.Ln)
        if "natural_log_exp_and_others" not in t:
            return t
        return {k: (fns if k == "natural_log_exp_and_others"
                    else fns - {exp, ln}) for k, fns in t.items()}

    bacc.get_activation_tables = steer
    bacc._act_tables_patched = True


def _bank_split(r0, r1):
    out = []
    while r0 < r1:
        nxt = min(r1, (r0 // 512 + 1) * 512)
        out.append((r0, nxt))
        r0 = nxt
    return out


def _build(los):
    dt = mybir.dt
    _patch_act_tables()
    nc = bacc.Bacc("TRN2", target_bir_lowering=False, debug=False,
                   num_devices=NCORES)
    et_win = nc.dram_tensor("et_win", [D, W], dt.bfloat16,
                            kind="ExternalInput").ap()
    et_y = nc.dram_tensor("et_y", [D, YROWS], dt.bfloat16,
                          kind="ExternalInput").ap()
    et_tr = nc.dram_tensor("et_tr", [D, TR], dt.bfloat16,
                           kind="ExternalInput").ap()
    et_own = nc.dram_tensor("et_own", [D, COLS], dt.bfloat16,
                            kind="ExternalInput").ap()
    oh_win = nc.dram_tensor("oh_win", [NCLS, W], dt.bfloat16,
                            kind="ExternalInput").ap()
    ohn_own = nc.dram_tensor("ohn_own", [NCLS, COLS], dt.bfloat16,
                             kind="ExternalInput").ap()
    out = nc.dram_tensor("out", [128, 2 * NCT], dt.float32,
                         kind="ExternalOutput").ap()
    scratch = nc.dram_tensor("scratch", [1, COLS], dt.float32,
                             kind="Internal").ap()

    with tile.TileContext(nc) as tc:
        with ExitStack() as ctx:
            cpool = ctx.enter_context(tc.tile_pool(name="consts", bufs=1))
            dpool = ctx.enter_context(tc.tile_pool(name="dumps", bufs=2))
            vpool = ctx.enter_context(tc.tile_pool(name="conv", bufs=2))
            ps_pool = ctx.enter_context(
                tc.tile_pool(name="psum", bufs=1, space="PSUM"))

            # tiny warm-up activation so the Exp+Ln ACT table set loads
            # during the input-DMA dead time (Ln forces natural_log_exp set)
            warm = cpool.tile([128, 1], dt.float32)
            nc.vector.memset(warm[:], 1.0)
            nc.scalar.activation(warm[:], warm[:],
                                 mybir.ActivationFunctionType.Ln)

            t_win = cpool.tile([D, W], dt.bfloat16)
            t_y = cpool.tile([D, YROWS], dt.bfloat16)
            t_tr = cpool.tile([D, TR], dt.bfloat16)
            t_own = cpool.tile([D, COLS], dt.bfloat16)
            t_oh = cpool.tile([NCLS, W], dt.bfloat16)
            t_ohn = cpool.tile([NCLS, COLS], dt.bfloat16)
            # first col-tile needs et_own[:, :128] + et_win: split those DMAs
            # so the first matmuls start as early as possible
            nc.sync.dma_start(t_own[:, 0:256], et_own[:, 0:256])
            nc.sync.dma_start(t_win[:, 0:512], et_win[:, 0:512])
            nc.sync.dma_start(t_win[:, 512:], et_win[:, 512:])
            nc.sync.dma_start(t_own[:, 256:], et_own[:, 256:])
            nc.gpsimd.dma_start(t_ohn[:], ohn_own[:])
            nc.gpsimd.dma_start(t_oh[:], oh_win[:])
            nc.sync.dma_start(t_y[:], et_y[:])
            for q in range(4):
                eng = nc.gpsimd if q >= 2 else nc.sync
                eng.dma_start(t_tr[:, q * (TR // 4):(q + 1) * (TR // 4)],
                              et_tr[:, q * (TR // 4):(q + 1) * (TR // 4)])

            ones = cpool.tile([128, 1], dt.bfloat16)
            nc.vector.memset(ones[:], 1.0)
            s_part = cpool.tile([128, NCT, 3], dt.float32)   # A, B, scat
            a_part = cpool.tile([128, NCT], dt.float32)
            s_sb = cpool.tile([128, NCT], dt.float32)
            m_sb = cpool.tile([128, NCT], dt.float32)
            rbig = cpool.tile([128, NCT, SW], dt.float32)
            ssb = cpool.tile([128, 512], dt.float32)

            big = ps_pool.tile([128, 4096], dt.float32)
            s_half = [big[0:1, PS:PS + 512], big[32:33, PS:PS + 512]]

            # The ones-matmuls for block k are emitted during block k+1, so
            # the PE FIFO never stalls waiting for block k's DVE convert.
            pending = []          # [(blk, rd0, rd1)]

            def emit_ones(blk, rd0, rd1):
                for h, rd in ((0, rd0), (1, rd1)):
                    nc.tensor.matmul(
                        s_half[h], ones[:], rd,
                        start=(blk == 0), stop=(blk == NBLK - 1),
                        skip_group_check=True)

            def trans_block(blk):
                lhs = t_tr[:, blk * 128:(blk + 1) * 128]
                on_act = (blk % ACT_BLK_EVERY) == (ACT_BLK_EVERY - 1)
                bufs = []
                for h, pt in ((0, PT0), (1, PT1)):
                    nc.tensor.matmul(big[:, pt:pt + 512], lhs,
                                     t_own[:, h * 512:(h + 1) * 512],
                                     start=True, stop=True,
                                     skip_group_check=True)
                if pending:
                    emit_ones(*pending.pop())
                for h, pt in ((0, PT0), (1, PT1)):
                    tbuf = big[:, pt:pt + 512]
                    if on_act:
                        cb = vpool.tile([128, 512], dt.bfloat16,
                                        tag=f"ab{h}")
                        nc.scalar.activation(
                            cb[:], tbuf, mybir.ActivationFunctionType.Exp,
                            scale=SCALE)
                        bufs.append(cb[:])
                    else:
                        cb = vpool.tile([128, 512], dt.int16, tag=f"cv{h}")
                        nc.vector.tensor_scalar(
                            out=cb[:], in0=tbuf, scalar1=K2, scalar2=B2,
                            op0=mybir.AluOpType.mult,
                            op1=mybir.AluOpType.add)
                        bufs.append(cb[:].bitcast(dt.bfloat16))
                pending.append((blk, bufs[0], bufs[1]))

            def emit_mln(ct):
                nc.vector.reduce_sum(s_sb[:, ct:ct + 1],
                                     s_part[:, ct:ct + 1, :],
                                     axis=mybir.AxisListType.X)
                nc.vector.tensor_scalar_mul(
                    m_sb[:, ct:ct + 1], s_sb[:, ct:ct + 1], MCONST)
                ld = dpool.tile([128, SW], dt.bfloat16, tag="lndump")
                nc.scalar.activation(
                    ld[:], rbig[:, ct, :], mybir.ActivationFunctionType.Ln,
                    scale=m_sb[:, ct:ct + 1], bias=1.0,
                    accum_out=a_part[:, ct:ct + 1])

            blk_sched = [8, 8, 8, 8, 8, 4, 0, 0]
            blk_iter = iter(range(NBLK))

            for ct in range(NCT):
                lo = los[ct]
                lhs_et = t_own[:, ct * 128:(ct + 1) * 128]
                lhs_oh = t_ohn[:, ct * 128:(ct + 1) * 128]

                # ---- column group A: window rows (masked slice) ----
                for r0, r1 in ((0, 512), (512, 1024), (1024, W)):
                    nc.tensor.matmul(big[:, PC + r0:PC + r1], lhs_et,
                                     t_win[:, r0:r1], start=True, stop=False,
                                     skip_group_check=True)
                for r0, r1 in _bank_split(lo, lo + SW):
                    nc.tensor.matmul(big[:, PC + r0:PC + r1], lhs_oh,
                                     t_oh[:, r0:r1], start=False, stop=True,
                                     skip_group_check=True)
                xad = dpool.tile([128, W], dt.float32, tag="xadump")
                nc.scalar.activation(
                    xad[:], big[:, PC:PC + W],
                    mybir.ActivationFunctionType.Exp, scale=SCALE,
                    accum_out=s_part[:, ct, 0:1])
                nc.vector.reciprocal_approx_fast(
                    out=rbig[:, ct, :], in_=xad[:, lo:lo + SW])

                for _ in range(blk_sched[ct] // 2):
                    trans_block(next(blk_iter))

                # ---- column group B: mainB rows (unmasked) ----
                for r0, r1 in _bank_split(W, W + YROWS):
                    nc.tensor.matmul(big[:, PC + r0:PC + r1], lhs_et,
                                     t_y[:, r0 - W:r1 - W],
                                     start=True, stop=True,
                                     skip_group_check=True)
                ybd = dpool.tile([128, YROWS], dt.bfloat16, tag="ybdump")
                nc.scalar.activation(
                    ybd[:], big[:, PC + W:PC + W + YROWS],
                    mybir.ActivationFunctionType.Exp, scale=SCALE,
                    accum_out=s_part[:, ct, 1:2])

                for _ in range(blk_sched[ct] - blk_sched[ct] // 2):
                    trans_block(next(blk_iter))

                if ct == 5:
                    # S halves -> scratch DRAM -> [128, 8] scatter
                    nc.vector.tensor_copy(ssb[:], big[:, PS:PS + 512])
                    nc.sync.dma_start(
                        scratch[:].rearrange("o (ph r) -> ph (o r)", ph=2),
                        ssb[:].rearrange("(a b) r -> a b r", b=32)[0:2, 0:1])
                    nc.sync.dma_start(
                        s_part[:, :, 2:3],
                        scratch[:].rearrange("o (c p) -> (o p) c", p=128))
                if ct == 6:
                    for c2 in range(6):
                        emit_mln(c2)
            emit_mln(6)
            emit_mln(7)

            nc.sync.dma_start(out[:, 0:NCT], a_part[:])
            nc.sync.dma_start(out[:, NCT:], s_sb[:])
    nc.compile()
    return nc


def _get_nc(los):
    key = ("v3", tuple(los))
    if key not in _cache:
        _cache[key] = _build(tuple(los))
    return _cache[key]


def _prepare(embeds, labels):
    embeds = np.ascontiguousarray(np.asarray(embeds, dtype=np.float32))
    labels_i = np.asarray(labels).astype(np.int64)
    assert embeds.shape == (B, D)

    perm = np.argsort(labels_i, kind="stable")
    lab = labels_i[perm]
    emb = embeds[perm]
    ebf = emb.astype(ml_dtypes.bfloat16)
    et = np.ascontiguousarray(ebf.T)                      # [D, B] bf16
    oh = np.zeros((NCLS, B), dtype=ml_dtypes.bfloat16)
    oh[lab, np.arange(B)] = ml_dtypes.bfloat16(1.0)
    ohn = oh * ml_dtypes.bfloat16(MASKVAL)

    starts = np.searchsorted(lab, np.arange(NCLS), side="left")
    ends = np.searchsorted(lab, np.arange(NCLS), side="right")

    s_min = [10**9] * NCT
    e_max = [0] * NCT
    spans = []
    for c in range(NCORES):
        lo, hi = c * COLS, (c + 1) * COLS
        r_lo = int(starts[lab[lo]])
        r_hi = int(ends[lab[hi - 1]])
        spans.append((r_lo, r_hi))
        assert r_hi - r_lo <= W, f"span overflow: {r_hi - r_lo}"
        for ct in range(NCT):
            cl, ch = lo + ct * 128, lo + (ct + 1) * 128
            s_ct = int(starts[lab[cl]]) - r_lo
            e_ct = int(ends[lab[ch - 1]]) - r_lo
            s_min[ct] = min(s_min[ct], s_ct)
            e_max[ct] = max(e_max[ct], e_ct)
    los = []
    for ct in range(NCT):
        lo_ct = max(0, min(e_max[ct] - SW, s_min[ct], W - SW))
        assert lo_ct <= s_min[ct] and e_max[ct] <= lo_ct + SW, (
            f"slice infeasible ct={ct}: [{s_min[ct]},{e_max[ct]}] "
            f"vs lo={lo_ct} SW={SW}")
        los.append(lo_ct)

    in_maps = []
    for c in range(NCORES):
        r_lo, r_hi = spans[c]
        span = r_hi - r_lo
        fill = W - span
        after = np.arange(r_hi, min(B, r_hi + fill))
        need = fill - len(after)
        before = np.arange(r_lo - need, r_lo) if need > 0 else np.arange(0)
        win_rows = np.concatenate([np.arange(r_lo, r_hi), after, before])
        assert len(win_rows) == W
        main_mask = np.ones(B, dtype=bool)
        main_mask[win_rows] = False
        main_idx = np.nonzero(main_mask)[0]
        assert len(main_idx) == YROWS + TR
        lo = c * COLS
        in_maps.append({
            "et_win": np.ascontiguousarray(et[:, win_rows]),
            "et_y": np.ascontiguousarray(et[:, main_idx[:YROWS]]),
            "et_tr": np.ascontiguousarray(et[:, main_idx[YROWS:]]),
            "et_own": np.ascontiguousarray(et[:, lo:lo + COLS]),
            "oh_win": np.ascontiguousarray(oh[:, win_rows]),
            "ohn_own": np.ascontiguousarray(ohn[:, lo:lo + COLS]),
        })
    return in_maps, los, lab, ebf


def _combine(results, lab, ebf):
    S = np.empty(B, dtype=np.float64)
    A = np.empty(B, dtype=np.float64)
    for c in range(NCORES):
        o = results[c]["out"].astype(np.float64)
        A[c * COLS:(c + 1) * COLS] = o[:, 0:NCT].T.reshape(-1)
        S[c * COLS:(c + 1) * COLS] = o[:, NCT:].T.reshape(-1)

    counts = np.bincount(lab, minlength=NCLS)
    count_j = counts[lab].astype(np.float64) - 1.0
    l_jj = (ebf.astype(np.float64) ** 2).sum(1) * SCALE
    u_jj = np.log1p(S * np.exp(EPS - l_jj))
    numer = A - EPS * count_j - u_jj
    loss = (numer / count_j).sum() / B
    return np.asarray(loss, dtype=np.float32)


def kernel(embeds, labels):
    in_maps, los, lab, ebf = _prepare(embeds, labels)
    nc = _get_nc(los)
    res = run_bass_kernel_spmd(nc, in_maps, list(range(NCORES)))
    return _combine(res.results, lab, ebf)
